# revision 30
# baseline (speedup 1.0000x reference)
"""Trainium2 Bass kernel for nn_CEN_BRL: context-encoder + LSTM + antecedent
attention scoring, distributed over 8 NeuronCores.

Structure (per the sharding strategy):
  - Phase A (train-sharded, 512 rows/core): encoder MLP -> x_proj, then the
    32-step LSTM. The recurrence is row-independent, so each core runs its own
    512 rows and accumulates per-step row-sums of h into esums[gh, t].
  - Phase B: two AllReduces (steps 0..15 / 16..31) of the esums chunks, then
    b_t = e_t @ W1_h + att_b1 on-device -> B matrix [128, 32] (duplicated
    halves for the packed attention layout).
  - Phase C (ante-sharded, 4096 antes/core): S_proj = S.T @ att_W1[:4096] via
    PE, packed as [128, 2048] (two ante-halves stacked on partitions) using
    half-zeroed M=128 stationaries that scatter each half into its partition
    range via PSUM accumulation. Then per step: relu(S_proj + b_t) split
    across DVE and ACT, and a w2 matvec on PE whose per-step block stationary
    scatters scores into rows (t | T+t) of a [2T, 512] PSUM accumulator.

Because Tile emits a static per-engine schedule, the S_proj matmul stream is
interleaved into the LSTM step loop (one S k-chunk per step): the S_proj and
next-step preload matmuls fill PE idle time while each step's h_t dependency
chain runs on ACT/DVE.

The LSTM runs in bf16 (PSUM accumulation fp32); S_proj/scores stay fp32 so the
argmax is robust. The final argmax over the gathered [32, 32768] scores is
done on host (exact, matches jnp.argmax first-max tie-breaking).
"""

import sys

if "/opt/trn_rl_repo" not in sys.path:
    sys.path.insert(0, "/opt/trn_rl_repo")

from contextlib import ExitStack

import ml_dtypes
import numpy as np

import concourse.bass as bass
import concourse.tile as tile
from concourse import bacc, mybir
from concourse.bass_utils import run_bass_kernel_spmd

F32 = mybir.dt.float32
F32R = mybir.dt.float32r
BF16 = mybir.dt.bfloat16
AF = mybir.ActivationFunctionType
ALU = mybir.AluOpType

N_CORES = 8
N_TRAIN, N_FEAT, N_HID, ENC = 4096, 64, 256, 256
GH, AH, N_ANTES, MAX_LEN = 256, 64, 32768, 32


def build_program(n_train=N_TRAIN, n_antes=N_ANTES, t_steps=MAX_LEN,
                  n_cores=N_CORES):
    """Emit the SPMD Bass program (identical on all cores; inputs differ)."""
    tpc = n_train // n_cores           # train rows per core
    apc = n_antes // n_cores           # antecedents per core
    kch = n_train // 128               # contraction chunks for S_proj
    jb = apc // 1024                   # 512-wide ante blocks per half
    half = apc // 2
    t_split = 3 * t_steps // 4
    csizes = [t_split, t_steps - t_split]

    nc = bacc.Bacc("TRN2", target_bir_lowering=False, debug=False,
                   num_devices=n_cores)

    # ---- I/O declarations ----
    ctxT = nc.dram_tensor("ctxT", [N_FEAT, tpc], F32R, kind="ExternalInput")
    S_in = nc.dram_tensor("S", [n_train, apc], F32R, kind="ExternalInput")
    encW1 = nc.dram_tensor("encW1", [N_FEAT, N_HID], F32R, kind="ExternalInput")
    encb1 = nc.dram_tensor("encb1", [128, 2], F32, kind="ExternalInput")
    encW2 = nc.dram_tensor("encW2", [128, 2, ENC], F32R, kind="ExternalInput")
    encb2 = nc.dram_tensor("encb2", [128, 2], F32, kind="ExternalInput")
    WihT = nc.dram_tensor("WihT", [128, 2, 4 * GH], F32R, kind="ExternalInput")
    bihh = nc.dram_tensor("bihh", [128, 8], F32, kind="ExternalInput")
    WhhT = nc.dram_tensor("WhhT", [128, 2, 4 * GH], BF16, kind="ExternalInput")
    ident = nc.dram_tensor("ident", [128, 128], BF16, kind="ExternalInput")
    # att_W1[:n_train] chunks, zero-padded per half: [:, kc, h, :] is [128,128]
    # with att_W1 chunk kc in columns h*64:(h+1)*64 and zeros elsewhere
    attW1z = nc.dram_tensor("attW1z", [128, kch, 2, 128], F32R,
                            kind="ExternalInput")
    W1hdup = nc.dram_tensor("W1hdup", [128, 2, 128], F32, kind="ExternalInput")
    attb1d = nc.dram_tensor("attb1d", [128, 1], F32, kind="ExternalInput")
    w2blk = nc.dram_tensor("w2blk", [128, t_steps, 128], F32R,
                           kind="ExternalInput")
    scores_out = nc.dram_tensor("scores", [t_steps, apc], F32,
                                kind="ExternalOutput")

    with tile.TileContext(nc) as tc, ExitStack() as ctx:
        consts = ctx.enter_context(tc.tile_pool(name="consts", bufs=1))
        dram = ctx.enter_context(tc.tile_pool(name="dram", bufs=1, space="DRAM"))

        # ---- load constants ----
        def cload(dten, shape, dtype):
            t = consts.tile(shape, dtype, tag=dten.name, name=dten.name + "_sb")
            nc.sync.dma_start(out=t[:], in_=dten[:])
            return t

        sb_encW1 = cload(encW1, [N_FEAT, N_HID], F32R)
        sb_encb1 = cload(encb1, [128, 2], F32)
        sb_encW2 = cload(encW2, [128, 2, ENC], F32R)
        sb_encb2 = cload(encb2, [128, 2], F32)
        sb_WihT = cload(WihT, [128, 2, 4 * GH], F32R)
        sb_bihh = cload(bihh, [128, 8], F32)
        sb_WhhT = cload(WhhT, [128, 2, 4 * GH], BF16)
        sb_ident = cload(ident, [128, 128], BF16)
        sb_attW1 = cload(attW1z, [128, kch, 2, 128], F32R)
        sb_W1hdup = cload(W1hdup, [128, 2, 128], F32)
        sb_attb1d = cload(attb1d, [128, 1], F32)
        sb_w2blk = cload(w2blk, [128, t_steps, 128], F32R)
        sb_ctxT = cload(ctxT, [N_FEAT, tpc], F32R)

        # persistent activations (free layout [8, tpc]: mtile m at [:, m, :])
        sb_xproj = consts.tile([128, 8, tpc], BF16, tag="xproj")
        sb_sproj = consts.tile([128, half], F32, tag="sproj")
        sb_esums = consts.tile([128, 2, t_steps], F32, tag="esums")
        sb_B = consts.tile([128, t_steps], F32, tag="Bmat")

        es_bounce = []

        # ---- S pools open first so S_proj k-chunks can fill encoder gaps ----
        with tc.tile_pool(name="s_pool", bufs=3) as s_pool, \
             tc.tile_pool(name="ps_sproj", bufs=1, space="PSUM") as ps_sproj:
          sp_ps = [ps_sproj.tile([128, 512], F32, tag=f"sp{j}",
                                 name=f"sp{j}") for j in range(jb)]

          def emit_sproj(kc):
              st = s_pool.tile([128, apc], F32R, tag="s_t", name="s_t")
              nc.sync.dma_start(out=st[:],
                                in_=S_in[kc * 128:(kc + 1) * 128, :])
              for j in range(jb):
                  for h in range(2):
                      # M=128 with half-zeroed stationary: half h lands in
                      # partitions h*64..h*64+64, zeros accumulate elsewhere
                      nc.tensor.matmul(
                          sp_ps[j][:], sb_attW1[:, kc, h, :],
                          st[:, h * half + j * 512:h * half + (j + 1) * 512],
                          start=(kc == 0 and h == 0),
                          stop=(kc == kch - 1 and h == 1))

          n_early = min(2, kch)
          rem = kch - n_early

          # ---- encoder: phi = relu(ctx@W1+b1)@W2+b2; x_proj = phi@WihT+b ----
          with tc.tile_pool(name="enc_sb", bufs=1) as enc_sb, \
               tc.tile_pool(name="ps_enc", bufs=2, space="PSUM") as ps_enc:
            sb_hid = enc_sb.tile([128, 2, tpc], F32R, tag="hidT")
            for mc in range(2):
                ps = ps_enc.tile([128, tpc], F32, tag="e", name="pse")
                nc.tensor.matmul(ps[:], sb_encW1[:, mc * 128:(mc + 1) * 128],
                                 sb_ctxT[:], start=True, stop=True)
                nc.scalar.activation(sb_hid[:, mc, :], ps[:], AF.Relu,
                                     bias=sb_encb1[:, mc:mc + 1])
            if n_early > 0:
                emit_sproj(0)
            sb_phi = enc_sb.tile([128, 2, tpc], F32R, tag="phiT")
            for ec in range(2):
                ps = ps_enc.tile([128, tpc], F32, tag="e", name="pse")
                for kc in range(2):
                    nc.tensor.matmul(ps[:],
                                     sb_encW2[:, kc, ec * 128:(ec + 1) * 128],
                                     sb_hid[:, kc, :],
                                     start=(kc == 0), stop=(kc == 1))
                nc.scalar.activation(sb_phi[:, ec, :], ps[:], AF.Identity,
                                     bias=sb_encb2[:, ec:ec + 1])
            if n_early > 1:
                emit_sproj(1)
            for m in range(8):
                ps = ps_enc.tile([128, tpc], F32, tag="e", name="pse")
                for kc in range(2):
                    nc.tensor.matmul(ps[:],
                                     sb_WihT[:, kc, m * 128:(m + 1) * 128],
                                     sb_phi[:, kc, :],
                                     start=(kc == 0), stop=(kc == 1))
                nc.scalar.activation(sb_xproj[:, m, :], ps[:], AF.Identity,
                                     bias=sb_bihh[:, m:m + 1])

          # ---- LSTM with the S_proj stream interleaved per step ----
          with tc.tile_pool(name="lstm", bufs=2) as lp, \
               tc.tile_pool(name="ps_gates", bufs=2, space="PSUM") as ps_g:
            h_prev = None
            c_prev = None
            fns = [AF.Sigmoid, AF.Sigmoid, AF.Tanh, AF.Sigmoid]
            for t in range(t_steps):
                gates = []
                for gp in range(2):              # gate pairs (0,1) and (2,3)
                    pair = (2 * gp, 2 * gp + 1)
                    psgs = []
                    for g in pair:
                        # one [128, 2, 512]-padded psum tile per gate: the two
                        # col-halves land in separate zero regions. The pair's
                        # x_proj preloads are emitted before any whh matmul so
                        # they fill the h_{t-1} dependency stall on PE.
                        psg = ps_g.tile([128, 2, tpc], F32, tag="g", name="psg",
                                        padded_shape=[128, 2, 512])
                        psgs.append(psg)
                        for m2 in range(2):
                            nc.tensor.matmul(psg[:, m2, :], sb_ident[:],
                                             sb_xproj[:, 2 * g + m2, :],
                                             start=True, stop=(t == 0))
                    if t > 0:
                        for g, psg in zip(pair, psgs):
                            for m2 in range(2):
                                m = 2 * g + m2
                                for kc in range(2):
                                    nc.tensor.matmul(
                                        psg[:, m2, :],
                                        sb_WhhT[:, kc, m * 128:(m + 1) * 128],
                                        h_prev[:, kc, :],
                                        start=False, stop=(kc == 1))
                    for g, psg in zip(pair, psgs):
                        gt_ = lp.tile([128, 2, tpc], BF16, tag=f"g{g}",
                                      name=f"g{g}")
                        nc.scalar.activation(gt_[:], psg[:], fns[g])
                        gates.append(gt_)
                gi, gf, gg, go = gates
                ig = lp.tile([128, 2, tpc], BF16, tag="ig")
                nc.vector.tensor_tensor(ig[:], gi[:], gg[:], op=ALU.mult)
                if t == 0:
                    c_new = ig
                else:
                    c_new = lp.tile([128, 2, tpc], BF16, tag="c")
                    fc = lp.tile([128, 2, tpc], BF16, tag="fc")
                    nc.vector.tensor_tensor(fc[:], gf[:], c_prev[:],
                                            op=ALU.mult)
                    nc.vector.tensor_tensor(c_new[:], fc[:], ig[:],
                                            op=ALU.add)
                tc_t = lp.tile([128, 2, tpc], BF16, tag="tc")
                nc.scalar.activation(tc_t[:], c_new[:], AF.Tanh)
                h_new = lp.tile([128, 2, tpc], BF16, tag="h")
                for kc in range(2):
                    # h = o * tanh(c); accum_out = train-row sum for e_t
                    nc.vector.scalar_tensor_tensor(
                        h_new[:, kc, :], go[:, kc, :], 0.0, tc_t[:, kc, :],
                        op0=ALU.add, op1=ALU.mult,
                        accum_out=sb_esums[:, kc, t:t + 1])
                h_prev, c_prev = h_new, c_new

                # interleave this step's share of the S_proj stream
                for k in range(rem):
                    if k * t_steps // rem == t:
                        emit_sproj(n_early + k)

                # esums chunk boundary: bounce to DRAM + AllReduce
                if t == t_split - 1 or t == t_steps - 1:
                    hchunk = 0 if t == t_split - 1 else 1
                    c0 = 0 if hchunk == 0 else t_split
                    csz = csizes[hchunk]
                    ein = dram.tile([128, 2, csz], F32,
                                    tag=f"ein{hchunk}", name=f"ein{hchunk}")
                    eout = dram.tile([128, 2, csz], F32,
                                     tag=f"eout{hchunk}", name=f"eout{hchunk}")
                    nc.sync.dma_start(
                        out=ein[:],
                        in_=sb_esums[:, :, c0:c0 + csz])
                    nc.gpsimd.collective_compute(
                        "AllReduce", ALU.add,
                        replica_groups=[list(range(n_cores))],
                        ins=[ein.opt()], outs=[eout.opt()])
                    es_sb = consts.tile([128, 2, csz], F32,
                                        tag=f"es{hchunk}", name=f"es{hchunk}")
                    nc.sync.dma_start(out=es_sb[:], in_=eout[:])
                    es_bounce.append(es_sb)

            for j in range(jb):
                nc.scalar.activation(sb_sproj[:, j * 512:(j + 1) * 512],
                                     sp_ps[j][:], AF.Copy)

        # ---- B matrix + attention scores, interleaved so the second
        # AllReduce hides behind the first t_split steps of phase C ----
        with tc.tile_pool(name="ps_b", bufs=1, space="PSUM") as ps_b, \
             tc.tile_pool(name="hid_pool", bufs=3) as hid_pool, \
             tc.tile_pool(name="ps_sc", bufs=1, space="PSUM") as ps_sc, \
             tc.tile_pool(name="sc_sb", bufs=1) as sc_sb:
            sc_ps = [ps_sc.tile([128, 512], F32, tag=f"sc{j}",
                                name=f"sc{j}") for j in range(jb)]
            hh = half // 2

            def emit_B(hchunk):
                c0 = 0 if hchunk == 0 else t_split
                csz = csizes[hchunk]
                psb = ps_b.tile([128, csz], F32, tag="b", name="psb")
                for kc in range(2):
                    nc.tensor.matmul(psb[:], sb_W1hdup[:, kc, :],
                                     es_bounce[hchunk][:, kc, :],
                                     start=(kc == 0), stop=(kc == 1))
                nc.scalar.activation(sb_B[:, c0:c0 + csz], psb[:],
                                     AF.Identity, bias=sb_attb1d[:])

            def emit_C(t):
                hid = hid_pool.tile([128, half], F32R, tag="hid", name="hid")
                # relu(S_proj + b_t): low cols on DVE, high cols on ACT
                nc.vector.tensor_scalar(hid[:, 0:hh], sb_sproj[:, 0:hh],
                                        sb_B[:, t:t + 1], 0.0,
                                        op0=ALU.add, op1=ALU.max)
                nc.scalar.activation(hid[:, hh:half], sb_sproj[:, hh:half],
                                     AF.Relu, bias=sb_B[:, t:t + 1])
                for j in range(jb):
                    # w2blk[:, t, :] scatters this step's scores into psum
                    # rows t (A-half) / T+t (B-half); other rows get +0
                    nc.tensor.matmul(sc_ps[j][:], sb_w2blk[:, t, :],
                                     hid[:, j * 512:(j + 1) * 512],
                                     start=(t == 0), stop=(t == t_steps - 1))

            emit_B(0)
            for t in range(t_split):
                emit_C(t)
            emit_B(1)
            for t in range(t_split, t_steps):
                emit_C(t)
            for j in range(jb):
                sb = sc_sb.tile([2 * t_steps, 512], F32, tag=f"scb{j}",
                                name=f"scb{j}")
                nc.scalar.activation(sb[:], sc_ps[j][0:2 * t_steps, :], AF.Copy)
                nc.sync.dma_start(out=scores_out[:, j * 512:(j + 1) * 512],
                                  in_=sb[0:t_steps, :])
                nc.sync.dma_start(out=scores_out[:, half + j * 512:
                                                 half + (j + 1) * 512],
                                  in_=sb[t_steps:2 * t_steps, :])

    nc.compile()
    return nc


def prep_inputs(context, S, enc_W1, enc_b1, enc_W2, enc_b2, W_ih, W_hh,
                b_ih, b_hh, att_W1, att_b1, att_w2,
                n_train, n_antes, t_steps, n_cores):
    """Host-side prep: slice/transpose/chunk weights into device layouts."""
    f32 = np.float32
    bf16 = ml_dtypes.bfloat16
    tpc = n_train // n_cores
    apc = n_antes // n_cores
    kch = n_train // 128

    ctxT = np.ascontiguousarray(context.T.astype(f32))          # [64, n_train]
    encb1 = np.ascontiguousarray(enc_b1.astype(f32).reshape(2, 128).T)
    encW2 = np.ascontiguousarray(
        enc_W2.astype(f32).reshape(2, 128, ENC).transpose(1, 0, 2))
    encb2 = np.ascontiguousarray(enc_b2.astype(f32).reshape(2, 128).T)
    WihT = np.ascontiguousarray(
        W_ih.T.astype(f32).reshape(2, 128, 4 * GH).transpose(1, 0, 2))
    bihh = np.ascontiguousarray(
        (b_ih + b_hh).astype(f32).reshape(8, 128).T)
    WhhT = np.ascontiguousarray(
        W_hh.T.astype(f32).reshape(2, 128, 4 * GH).transpose(1, 0, 2)
    ).astype(bf16)
    identm = np.eye(128, dtype=f32).astype(bf16)
    attW1c = att_W1[:n_train].astype(f32).reshape(kch, 128, AH)
    attW1z = np.zeros((128, kch, 2, 128), f32)
    for kc in range(kch):
        attW1z[:, kc, 0, 0:64] = attW1c[kc]
        attW1z[:, kc, 1, 64:128] = attW1c[kc]
    V = att_W1[n_train:].astype(f32) / f32(n_train)             # [256, 64]
    W1hdup = np.ascontiguousarray(
        np.concatenate([V, V], axis=1).reshape(2, 128, 128).transpose(1, 0, 2))
    attb1d = np.ascontiguousarray(
        np.concatenate([att_b1, att_b1]).astype(f32).reshape(128, 1))
    w2blk = np.zeros((128, t_steps, 128), f32)
    for t in range(t_steps):
        w2blk[0:64, t, t] = att_w2.astype(f32)                  # A-half -> row t
        w2blk[64:128, t, t_steps + t] = att_w2.astype(f32)      # B-half -> T+t

    shared = dict(encW1=enc_W1.astype(f32), encb1=encb1, encW2=encW2,
                  encb2=encb2, WihT=WihT, bihh=bihh, WhhT=WhhT, ident=identm,
                  attW1z=attW1z, W1hdup=W1hdup, attb1d=attb1d, w2blk=w2blk)
    in_maps = []
    for c in range(n_cores):
        m = dict(shared)
        m["ctxT"] = np.ascontiguousarray(ctxT[:, c * tpc:(c + 1) * tpc])
        m["S"] = np.ascontiguousarray(S[:, c * apc:(c + 1) * apc].astype(f32))
        in_maps.append(m)
    return in_maps


_CACHED_NC = None


def kernel(context, S, enc_W1, enc_b1, enc_W2, enc_b2, W_ih, W_hh, b_ih, b_hh,
           att_W1, att_b1, att_w2, att_b2, max_len, _trace=False):
    global _CACHED_NC
    context = np.asarray(context)
    S = np.asarray(S)
    assert int(max_len) == MAX_LEN and S.shape == (N_TRAIN, N_ANTES)

    if _CACHED_NC is None:
        _CACHED_NC = build_program()
    nc = _CACHED_NC

    in_maps = prep_inputs(context, S, np.asarray(enc_W1), np.asarray(enc_b1),
                          np.asarray(enc_W2), np.asarray(enc_b2),
                          np.asarray(W_ih), np.asarray(W_hh),
                          np.asarray(b_ih), np.asarray(b_hh),
                          np.asarray(att_W1), np.asarray(att_b1),
                          np.asarray(att_w2), N_TRAIN, N_ANTES, MAX_LEN,
                          N_CORES)
    res = run_bass_kernel_spmd(nc, in_maps, list(range(N_CORES)), trace=_trace)
    apc = N_ANTES // N_CORES
    scores = np.empty((MAX_LEN, N_ANTES), np.float32)
    for c in range(N_CORES):
        scores[:, c * apc:(c + 1) * apc] = res.results[c]["scores"]
    scores += np.float32(np.asarray(att_b2).reshape(-1)[0])
    idx = np.argmax(scores, axis=-1).astype(np.int32)
    if _trace:
        kernel._last_results = res
    return scores, idx


# revision 31
# speedup vs baseline: 1.0216x; 1.0216x over previous
"""Trainium2 Bass kernel for nn_CEN_BRL: context-encoder + LSTM + antecedent
attention scoring, distributed over 8 NeuronCores.

Structure (per the sharding strategy):
  - Phase A (train-sharded, 512 rows/core): encoder MLP -> x_proj, then the
    32-step LSTM. The recurrence is row-independent, so each core runs its own
    512 rows and accumulates per-step row-sums of h into esums[gh, t].
  - Phase B: two AllReduces (steps 0..15 / 16..31) of the esums chunks, then
    b_t = e_t @ W1_h + att_b1 on-device -> B matrix [128, 32] (duplicated
    halves for the packed attention layout).
  - Phase C (ante-sharded, 4096 antes/core): S_proj = S.T @ att_W1[:4096] via
    PE, packed as [128, 2048] (two ante-halves stacked on partitions) using
    half-zeroed M=128 stationaries that scatter each half into its partition
    range via PSUM accumulation. Then per step: relu(S_proj + b_t) split
    across DVE and ACT, and a w2 matvec on PE whose per-step block stationary
    scatters scores into rows (t | T+t) of a [2T, 512] PSUM accumulator.

Because Tile emits a static per-engine schedule, the S_proj matmul stream is
interleaved into the LSTM step loop (one S k-chunk per step): the S_proj and
next-step preload matmuls fill PE idle time while each step's h_t dependency
chain runs on ACT/DVE.

The LSTM runs in bf16 (PSUM accumulation fp32); S_proj/scores stay fp32 so the
argmax is robust. The final argmax over the gathered [32, 32768] scores is
done on host (exact, matches jnp.argmax first-max tie-breaking).
"""

import sys

if "/opt/trn_rl_repo" not in sys.path:
    sys.path.insert(0, "/opt/trn_rl_repo")

from contextlib import ExitStack

import ml_dtypes
import numpy as np

import concourse.bass as bass
import concourse.tile as tile
from concourse import bacc, mybir
from concourse.bass_utils import run_bass_kernel_spmd

F32 = mybir.dt.float32
F32R = mybir.dt.float32r
BF16 = mybir.dt.bfloat16
AF = mybir.ActivationFunctionType
ALU = mybir.AluOpType

N_CORES = 8
N_TRAIN, N_FEAT, N_HID, ENC = 4096, 64, 256, 256
GH, AH, N_ANTES, MAX_LEN = 256, 64, 32768, 32


def build_program(n_train=N_TRAIN, n_antes=N_ANTES, t_steps=MAX_LEN,
                  n_cores=N_CORES):
    """Emit the SPMD Bass program (identical on all cores; inputs differ)."""
    tpc = n_train // n_cores           # train rows per core
    apc = n_antes // n_cores           # antecedents per core
    kch = n_train // 128               # contraction chunks for S_proj
    jb = apc // 1024                   # 512-wide ante blocks per half
    half = apc // 2
    t_split = 3 * t_steps // 4
    csizes = [t_split, t_steps - t_split]

    nc = bacc.Bacc("TRN2", target_bir_lowering=False, debug=False,
                   num_devices=n_cores)

    # ---- I/O declarations ----
    ctxT = nc.dram_tensor("ctxT", [N_FEAT, tpc], F32R, kind="ExternalInput")
    S_in = nc.dram_tensor("S", [n_train, apc], F32R, kind="ExternalInput")
    encW1 = nc.dram_tensor("encW1", [N_FEAT, N_HID], F32R, kind="ExternalInput")
    encb1 = nc.dram_tensor("encb1", [128, 2], F32, kind="ExternalInput")
    encW2 = nc.dram_tensor("encW2", [128, 2, ENC], F32R, kind="ExternalInput")
    encb2 = nc.dram_tensor("encb2", [128, 2], F32, kind="ExternalInput")
    WihT = nc.dram_tensor("WihT", [128, 2, 4 * GH], F32R, kind="ExternalInput")
    bihh = nc.dram_tensor("bihh", [128, 8], F32, kind="ExternalInput")
    WhhT = nc.dram_tensor("WhhT", [128, 2, 4 * GH], BF16, kind="ExternalInput")
    ident = nc.dram_tensor("ident", [128, 128], BF16, kind="ExternalInput")
    # att_W1[:n_train] chunks, zero-padded per half: [:, kc, h, :] is [128,128]
    # with att_W1 chunk kc in columns h*64:(h+1)*64 and zeros elsewhere
    attW1z = nc.dram_tensor("attW1z", [128, kch, 2, 128], F32R,
                            kind="ExternalInput")
    W1hdup = nc.dram_tensor("W1hdup", [128, 2, 128], F32, kind="ExternalInput")
    attb1d = nc.dram_tensor("attb1d", [128, 1], F32, kind="ExternalInput")
    w2blk = nc.dram_tensor("w2blk", [128, t_steps, 128], F32R,
                           kind="ExternalInput")
    scores_out = nc.dram_tensor("scores", [t_steps, apc], F32,
                                kind="ExternalOutput")

    with tile.TileContext(nc) as tc, ExitStack() as ctx:
        consts = ctx.enter_context(tc.tile_pool(name="consts", bufs=1))
        dram = ctx.enter_context(tc.tile_pool(name="dram", bufs=1, space="DRAM"))

        # ---- load constants ----
        def cload(dten, shape, dtype):
            t = consts.tile(shape, dtype, tag=dten.name, name=dten.name + "_sb")
            nc.sync.dma_start(out=t[:], in_=dten[:])
            return t

        sb_encW1 = cload(encW1, [N_FEAT, N_HID], F32R)
        sb_encb1 = cload(encb1, [128, 2], F32)
        sb_encW2 = cload(encW2, [128, 2, ENC], F32R)
        sb_encb2 = cload(encb2, [128, 2], F32)
        sb_WihT = cload(WihT, [128, 2, 4 * GH], F32R)
        sb_bihh = cload(bihh, [128, 8], F32)
        sb_WhhT = cload(WhhT, [128, 2, 4 * GH], BF16)
        sb_ident = cload(ident, [128, 128], BF16)
        sb_attW1 = cload(attW1z, [128, kch, 2, 128], F32R)
        sb_W1hdup = cload(W1hdup, [128, 2, 128], F32)
        sb_attb1d = cload(attb1d, [128, 1], F32)
        sb_w2blk = cload(w2blk, [128, t_steps, 128], F32R)
        sb_ctxT = cload(ctxT, [N_FEAT, tpc], F32R)

        # persistent activations (free layout [8, tpc]: mtile m at [:, m, :])
        sb_xproj = consts.tile([128, 8, tpc], BF16, tag="xproj")
        sb_sproj = consts.tile([128, half], F32, tag="sproj")
        sb_esums = consts.tile([128, 2, t_steps], F32, tag="esums")
        sb_B = consts.tile([128, t_steps], F32, tag="Bmat")

        es_bounce = []

        # ---- S pools open first so S_proj k-chunks can fill encoder gaps ----
        with tc.tile_pool(name="s_pool", bufs=3) as s_pool, \
             tc.tile_pool(name="ps_sproj", bufs=1, space="PSUM") as ps_sproj:
          sp_ps = [ps_sproj.tile([128, 512], F32, tag=f"sp{j}",
                                 name=f"sp{j}") for j in range(jb)]

          def emit_sproj(kc):
              st = s_pool.tile([128, apc], F32R, tag="s_t", name="s_t")
              nc.sync.dma_start(out=st[:],
                                in_=S_in[kc * 128:(kc + 1) * 128, :])
              for j in range(jb):
                  for h in range(2):
                      # M=128 with half-zeroed stationary: half h lands in
                      # partitions h*64..h*64+64, zeros accumulate elsewhere
                      nc.tensor.matmul(
                          sp_ps[j][:], sb_attW1[:, kc, h, :],
                          st[:, h * half + j * 512:h * half + (j + 1) * 512],
                          start=(kc == 0 and h == 0),
                          stop=(kc == kch - 1 and h == 1))

          n_early = min(2, kch)
          rem = kch - n_early

          # ---- encoder: phi = relu(ctx@W1+b1)@W2+b2; x_proj = phi@WihT+b ----
          with tc.tile_pool(name="enc_sb", bufs=1) as enc_sb, \
               tc.tile_pool(name="ps_enc", bufs=2, space="PSUM") as ps_enc:
            sb_hid = enc_sb.tile([128, 2, tpc], F32R, tag="hidT")
            for mc in range(2):
                ps = ps_enc.tile([128, tpc], F32, tag="e", name="pse")
                nc.tensor.matmul(ps[:], sb_encW1[:, mc * 128:(mc + 1) * 128],
                                 sb_ctxT[:], start=True, stop=True)
                nc.scalar.activation(sb_hid[:, mc, :], ps[:], AF.Relu,
                                     bias=sb_encb1[:, mc:mc + 1])
            if n_early > 0:
                emit_sproj(0)
            sb_phi = enc_sb.tile([128, 2, tpc], F32R, tag="phiT")
            for ec in range(2):
                ps = ps_enc.tile([128, tpc], F32, tag="e", name="pse")
                for kc in range(2):
                    nc.tensor.matmul(ps[:],
                                     sb_encW2[:, kc, ec * 128:(ec + 1) * 128],
                                     sb_hid[:, kc, :],
                                     start=(kc == 0), stop=(kc == 1))
                nc.scalar.activation(sb_phi[:, ec, :], ps[:], AF.Identity,
                                     bias=sb_encb2[:, ec:ec + 1])
            if n_early > 1:
                emit_sproj(1)
            for m in range(8):
                ps = ps_enc.tile([128, tpc], F32, tag="e", name="pse")
                for kc in range(2):
                    nc.tensor.matmul(ps[:],
                                     sb_WihT[:, kc, m * 128:(m + 1) * 128],
                                     sb_phi[:, kc, :],
                                     start=(kc == 0), stop=(kc == 1))
                nc.scalar.activation(sb_xproj[:, m, :], ps[:], AF.Identity,
                                     bias=sb_bihh[:, m:m + 1])

          # ---- LSTM with the S_proj stream interleaved per step ----
          with tc.tile_pool(name="lstm", bufs=2) as lp, \
               tc.tile_pool(name="ps_gates", bufs=2, space="PSUM") as ps_g:
            h_prev = None
            c_prev = None
            for t in range(t_steps):
                gates = []
                for g, fn in enumerate([AF.Sigmoid, AF.Sigmoid, AF.Tanh,
                                        AF.Sigmoid]):
                    # one [128, 2, 512]-padded psum tile per gate: the two
                    # gh-chunk col-halves land in separate zero regions
                    psg = ps_g.tile([128, 2, tpc], F32, tag="g", name="psg",
                                    padded_shape=[128, 2, 512])
                    for m2 in range(2):
                        m = 2 * g + m2
                        nc.tensor.matmul(psg[:, m2, :], sb_ident[:],
                                         sb_xproj[:, m, :],
                                         start=True, stop=(t == 0))
                        if t > 0:
                            for kc in range(2):
                                nc.tensor.matmul(
                                    psg[:, m2, :],
                                    sb_WhhT[:, kc, m * 128:(m + 1) * 128],
                                    h_prev[:, kc, :],
                                    start=False, stop=(kc == 1))
                    gt_ = lp.tile([128, 2, tpc], BF16, tag=f"g{g}",
                                  name=f"g{g}")
                    nc.scalar.activation(gt_[:], psg[:], fn)
                    gates.append(gt_)
                gi, gf, gg, go = gates
                ig = lp.tile([128, 2, tpc], BF16, tag="ig")
                nc.vector.tensor_tensor(ig[:], gi[:], gg[:], op=ALU.mult)
                if t == 0:
                    c_new = ig
                else:
                    c_new = lp.tile([128, 2, tpc], BF16, tag="c")
                    fc = lp.tile([128, 2, tpc], BF16, tag="fc")
                    nc.vector.tensor_tensor(fc[:], gf[:], c_prev[:],
                                            op=ALU.mult)
                    nc.vector.tensor_tensor(c_new[:], fc[:], ig[:],
                                            op=ALU.add)
                tc_t = lp.tile([128, 2, tpc], BF16, tag="tc")
                nc.scalar.activation(tc_t[:], c_new[:], AF.Tanh)
                h_new = lp.tile([128, 2, tpc], BF16, tag="h")
                for kc in range(2):
                    # h = o * tanh(c); accum_out = train-row sum for e_t
                    nc.vector.scalar_tensor_tensor(
                        h_new[:, kc, :], go[:, kc, :], 0.0, tc_t[:, kc, :],
                        op0=ALU.add, op1=ALU.mult,
                        accum_out=sb_esums[:, kc, t:t + 1])
                h_prev, c_prev = h_new, c_new

                # interleave this step's share of the S_proj stream
                for k in range(rem):
                    if k * t_steps // rem == t:
                        emit_sproj(n_early + k)

                # esums chunk boundary: bounce to DRAM + AllReduce
                if t == t_split - 1 or t == t_steps - 1:
                    hchunk = 0 if t == t_split - 1 else 1
                    c0 = 0 if hchunk == 0 else t_split
                    csz = csizes[hchunk]
                    ein = dram.tile([128, 2, csz], F32,
                                    tag=f"ein{hchunk}", name=f"ein{hchunk}")
                    eout = dram.tile([128, 2, csz], F32,
                                     tag=f"eout{hchunk}", name=f"eout{hchunk}")
                    nc.sync.dma_start(
                        out=ein[:],
                        in_=sb_esums[:, :, c0:c0 + csz])
                    nc.gpsimd.collective_compute(
                        "AllReduce", ALU.add,
                        replica_groups=[list(range(n_cores))],
                        ins=[ein.opt()], outs=[eout.opt()])
                    es_sb = consts.tile([128, 2, csz], F32,
                                        tag=f"es{hchunk}", name=f"es{hchunk}")
                    nc.sync.dma_start(out=es_sb[:], in_=eout[:])
                    es_bounce.append(es_sb)

            for j in range(jb):
                nc.scalar.activation(sb_sproj[:, j * 512:(j + 1) * 512],
                                     sp_ps[j][:], AF.Copy)

        # ---- B matrix + attention scores, interleaved so the second
        # AllReduce hides behind the first t_split steps of phase C ----
        with tc.tile_pool(name="ps_b", bufs=1, space="PSUM") as ps_b, \
             tc.tile_pool(name="hid_pool", bufs=3) as hid_pool, \
             tc.tile_pool(name="ps_sc", bufs=1, space="PSUM") as ps_sc, \
             tc.tile_pool(name="sc_sb", bufs=1) as sc_sb:
            sc_ps = [ps_sc.tile([128, 512], F32, tag=f"sc{j}",
                                name=f"sc{j}") for j in range(jb)]
            hh = half // 2

            def emit_B(hchunk):
                c0 = 0 if hchunk == 0 else t_split
                csz = csizes[hchunk]
                psb = ps_b.tile([128, csz], F32, tag="b", name="psb")
                for kc in range(2):
                    nc.tensor.matmul(psb[:], sb_W1hdup[:, kc, :],
                                     es_bounce[hchunk][:, kc, :],
                                     start=(kc == 0), stop=(kc == 1))
                nc.scalar.activation(sb_B[:, c0:c0 + csz], psb[:],
                                     AF.Identity, bias=sb_attb1d[:])

            def emit_C(t):
                hid = hid_pool.tile([128, half], F32R, tag="hid", name="hid")
                # relu(S_proj + b_t): low cols on DVE, high cols on ACT
                nc.vector.tensor_scalar(hid[:, 0:hh], sb_sproj[:, 0:hh],
                                        sb_B[:, t:t + 1], 0.0,
                                        op0=ALU.add, op1=ALU.max)
                nc.scalar.activation(hid[:, hh:half], sb_sproj[:, hh:half],
                                     AF.Relu, bias=sb_B[:, t:t + 1])
                for j in range(jb):
                    # w2blk[:, t, :] scatters this step's scores into psum
                    # rows t (A-half) / T+t (B-half); other rows get +0
                    nc.tensor.matmul(sc_ps[j][:], sb_w2blk[:, t, :],
                                     hid[:, j * 512:(j + 1) * 512],
                                     start=(t == 0), stop=(t == t_steps - 1))

            emit_B(0)
            for t in range(t_split):
                emit_C(t)
            emit_B(1)
            for t in range(t_split, t_steps):
                emit_C(t)
            for j in range(jb):
                sb = sc_sb.tile([2 * t_steps, 512], F32, tag=f"scb{j}",
                                name=f"scb{j}")
                nc.scalar.activation(sb[:], sc_ps[j][0:2 * t_steps, :], AF.Copy)
                nc.sync.dma_start(out=scores_out[:, j * 512:(j + 1) * 512],
                                  in_=sb[0:t_steps, :])
                nc.sync.dma_start(out=scores_out[:, half + j * 512:
                                                 half + (j + 1) * 512],
                                  in_=sb[t_steps:2 * t_steps, :])

    nc.compile()
    return nc


def prep_inputs(context, S, enc_W1, enc_b1, enc_W2, enc_b2, W_ih, W_hh,
                b_ih, b_hh, att_W1, att_b1, att_w2,
                n_train, n_antes, t_steps, n_cores):
    """Host-side prep: slice/transpose/chunk weights into device layouts."""
    f32 = np.float32
    bf16 = ml_dtypes.bfloat16
    tpc = n_train // n_cores
    apc = n_antes // n_cores
    kch = n_train // 128

    ctxT = np.ascontiguousarray(context.T.astype(f32))          # [64, n_train]
    encb1 = np.ascontiguousarray(enc_b1.astype(f32).reshape(2, 128).T)
    encW2 = np.ascontiguousarray(
        enc_W2.astype(f32).reshape(2, 128, ENC).transpose(1, 0, 2))
    encb2 = np.ascontiguousarray(enc_b2.astype(f32).reshape(2, 128).T)
    WihT = np.ascontiguousarray(
        W_ih.T.astype(f32).reshape(2, 128, 4 * GH).transpose(1, 0, 2))
    bihh = np.ascontiguousarray(
        (b_ih + b_hh).astype(f32).reshape(8, 128).T)
    WhhT = np.ascontiguousarray(
        W_hh.T.astype(f32).reshape(2, 128, 4 * GH).transpose(1, 0, 2)
    ).astype(bf16)
    identm = np.eye(128, dtype=f32).astype(bf16)
    attW1c = att_W1[:n_train].astype(f32).reshape(kch, 128, AH)
    attW1z = np.zeros((128, kch, 2, 128), f32)
    for kc in range(kch):
        attW1z[:, kc, 0, 0:64] = attW1c[kc]
        attW1z[:, kc, 1, 64:128] = attW1c[kc]
    V = att_W1[n_train:].astype(f32) / f32(n_train)             # [256, 64]
    W1hdup = np.ascontiguousarray(
        np.concatenate([V, V], axis=1).reshape(2, 128, 128).transpose(1, 0, 2))
    attb1d = np.ascontiguousarray(
        np.concatenate([att_b1, att_b1]).astype(f32).reshape(128, 1))
    w2blk = np.zeros((128, t_steps, 128), f32)
    for t in range(t_steps):
        w2blk[0:64, t, t] = att_w2.astype(f32)                  # A-half -> row t
        w2blk[64:128, t, t_steps + t] = att_w2.astype(f32)      # B-half -> T+t

    shared = dict(encW1=enc_W1.astype(f32), encb1=encb1, encW2=encW2,
                  encb2=encb2, WihT=WihT, bihh=bihh, WhhT=WhhT, ident=identm,
                  attW1z=attW1z, W1hdup=W1hdup, attb1d=attb1d, w2blk=w2blk)
    in_maps = []
    for c in range(n_cores):
        m = dict(shared)
        m["ctxT"] = np.ascontiguousarray(ctxT[:, c * tpc:(c + 1) * tpc])
        m["S"] = np.ascontiguousarray(S[:, c * apc:(c + 1) * apc].astype(f32))
        in_maps.append(m)
    return in_maps


_CACHED_NC = None


def kernel(context, S, enc_W1, enc_b1, enc_W2, enc_b2, W_ih, W_hh, b_ih, b_hh,
           att_W1, att_b1, att_w2, att_b2, max_len, _trace=False):
    global _CACHED_NC
    context = np.asarray(context)
    S = np.asarray(S)
    assert int(max_len) == MAX_LEN and S.shape == (N_TRAIN, N_ANTES)

    if _CACHED_NC is None:
        _CACHED_NC = build_program()
    nc = _CACHED_NC

    in_maps = prep_inputs(context, S, np.asarray(enc_W1), np.asarray(enc_b1),
                          np.asarray(enc_W2), np.asarray(enc_b2),
                          np.asarray(W_ih), np.asarray(W_hh),
                          np.asarray(b_ih), np.asarray(b_hh),
                          np.asarray(att_W1), np.asarray(att_b1),
                          np.asarray(att_w2), N_TRAIN, N_ANTES, MAX_LEN,
                          N_CORES)
    res = run_bass_kernel_spmd(nc, in_maps, list(range(N_CORES)), trace=_trace)
    apc = N_ANTES // N_CORES
    scores = np.empty((MAX_LEN, N_ANTES), np.float32)
    for c in range(N_CORES):
        scores[:, c * apc:(c + 1) * apc] = res.results[c]["scores"]
    scores += np.float32(np.asarray(att_b2).reshape(-1)[0])
    idx = np.argmax(scores, axis=-1).astype(np.int32)
    if _trace:
        kernel._last_results = res
    return scores, idx
